# revision 17
# baseline (speedup 1.0000x reference)
"""Trainium2 Bass kernel for nn_AddModelWithAttentionStacked.

Sharding (8 cores): core c handles batch b=c//2 and token-half h=c%2
(tokens [h*256, h*256+256) of L=512). The 6-layer attention stack runs
with per-pair AllGather of the updated xsa half each layer. The vocab
head (G=32000 logsumexp) is split across the pair (16000 columns each),
combined with a tiny stats AllGather. Per-batch results are read from
core 2b.

Token order on each core is LOCAL: [own half | other half]. Attention
and the head contractions are permutation-invariant over tokens, and the
roll(+-1) edge columns land at uniform local positions (511 / 256), so
one SPMD program serves all cores; the only data-dependent part is a
dynamic DMA offset (derived from partition_id) selecting the peer half
from the AllGather output.
"""

import numpy as np
import ml_dtypes

import concourse.bass as bass
import concourse.mybir as mybir
import concourse.tile as tile
from concourse import bacc
from concourse.bass_utils import run_bass_kernel_spmd
from concourse.masks import make_identity

bf16 = ml_dtypes.bfloat16
F32 = mybir.dt.float32
F32R = mybir.dt.float32r
BF = mybir.dt.bfloat16
I32 = mybir.dt.int32

P = 128
B, L, E, K, D, G, LM, KN = 4, 512, 256, 8, 6, 32000, 64, 4
R = L // 2          # own rows per core
GH = G // 2         # vocab half per core
KE = K * E          # 2048
NT = R // P         # 2  own-token tiles
MC = L // P         # 4  full-token tiles (local order)
EC = E // P         # 2  feature chunks
KC = KE // P        # 16 ke chunks
GC = 32             # vocab chunks per core
GCW = GH // GC      # 500 columns per vocab chunk
STEP = 0.05
Exp = mybir.ActivationFunctionType.Exp
Ln = mybir.ActivationFunctionType.Ln
Sqrt = mybir.ActivationFunctionType.Sqrt
Ident = mybir.ActivationFunctionType.Identity
Relu = mybir.ActivationFunctionType.Relu
Copy = mybir.ActivationFunctionType.Copy
ADD = mybir.AluOpType.add
MULT = mybir.AluOpType.mult

import os
N_LAYERS = int(os.environ.get("KERNEL_LAYERS", D))  # dev knob
STAGE = int(os.environ.get("KERNEL_STAGE", 99))  # truncate build for HW bisect


def _build():
    nc = bacc.Bacc("TRN2", target_bir_lowering=False, debug=False,
                   enable_asserts=False, num_devices=8)

    # ---------------- inputs (per-core) ----------------
    emb = nc.dram_tensor("emb", [G, E], F32, kind="ExternalInput")
    embT = nc.dram_tensor("embT", [P, EC, GH], BF, kind="ExternalInput")
    wdt = nc.dram_tensor("wdt", [D, P, KC, KE], BF, kind="ExternalInput")
    wqt = nc.dram_tensor("wqt", [D, P, EC, KE], BF, kind="ExternalInput")
    wov = nc.dram_tensor("wov", [D, P, KC, E], BF, kind="ExternalInput")
    wts = nc.dram_tensor("wts", [D, P, EC, E], BF, kind="ExternalInput")
    wtts = nc.dram_tensor("wtts", [D, P, EC, E], BF, kind="ExternalInput")
    wtcs = nc.dram_tensor("wtcs", [D, P, EC, E], BF, kind="ExternalInput")
    wtcts = nc.dram_tensor("wtcts", [D, P, EC, E], BF, kind="ExternalInput")
    wuts = nc.dram_tensor("wuts", [D, P, EC, E], BF, kind="ExternalInput")
    bts = nc.dram_tensor("bts", [D, 1, E], BF, kind="ExternalInput")
    wkct = nc.dram_tensor("wkct", [P, EC, KN * E], BF, kind="ExternalInput")
    bkcr = nc.dram_tensor("bkcr", [1, KN * E], BF, kind="ExternalInput")
    wem = nc.dram_tensor("wem", [P, EC, E], BF, kind="ExternalInput")
    zidx = nc.dram_tensor("zidx", [L, 1], I32, kind="ExternalInput")
    mrow = nc.dram_tensor("mrow", [LM, 1], I32, kind="ExternalInput")
    tgtr = nc.dram_tensor("tgtr", [LM, 1], I32, kind="ExternalInput")
    imaskd = nc.dram_tensor("imaskd", [P, LM], F32, kind="ExternalInput")

    outv = nc.dram_tensor("out", [1, 1], F32, kind="ExternalOutput")

    # internal DRAM for collectives (2 alternating sets)
    cc_in = [nc.dram_tensor(f"cc_in{i}", [R, E], BF) for i in range(2)]
    cc_out = [nc.dram_tensor(f"cc_out{i}", [L, E], BF) for i in range(2)]
    st_in = nc.dram_tensor("st_in", [R, 1], F32)
    st_out = nc.dram_tensor("st_out", [L, 1], F32)
    groups = [[0, 1], [2, 3], [4, 5], [6, 7]]

    with tile.TileContext(nc) as tc:
        with (
            tc.tile_pool(name="cst", bufs=1) as cst,
            tc.tile_pool(name="wsm", bufs=2) as wsm,      # small weights
            tc.tile_pool(name="wbig", bufs=2) as wbig,    # wq/wo
            tc.tile_pool(name="wd", bufs=2) as wdp,       # wd chunks
            tc.tile_pool(name="state", bufs=2) as stp,    # xsa tiles
            tc.tile_pool(name="act", bufs=1) as actp,     # per-layer activations
            tc.tile_pool(name="sc", bufs=2) as scp,       # small scratch
            tc.tile_pool(name="pt", bufs=2, space="PSUM") as pst,   # [128,512] transient
            tc.tile_pool(name="ph", bufs=2, space="PSUM") as psh,   # [128,256] held
            tc.tile_pool(name="py", bufs=2, space="PSUM") as psy,   # [128,512] values
            tc.tile_pool(name="pr", bufs=2, space="PSUM") as psr,   # R / S
        ):
            # ---- constants ----
            ident_bf = cst.tile([P, P], BF, tag="ident")
            make_identity(nc, ident_bf[:])
            ones_col_bf = cst.tile([P, 1], BF, tag="onescb")
            nc.vector.memset(ones_col_bf[:], 1.0)
            ones_col_f = cst.tile([P, 1], F32, tag="onescf")
            nc.vector.memset(ones_col_f[:], 1.0)
            ones_row_f = cst.tile([1, P], F32, tag="onesrf")
            nc.vector.memset(ones_row_f[:], 1.0)
            ones_row_bf = cst.tile([1, P], BF, tag="onesrb")
            nc.vector.memset(ones_row_bf[:], 1.0)
            imask = cst.tile([P, LM], F32, tag="imask")
            nc.sync.dma_start(imask[:], imaskd[:])

            # index tensors to SBUF
            zidx_s = cst.tile([P, MC], I32, tag="zidx")
            nc.sync.dma_start(
                zidx_s[:], zidx.rearrange("(mc p) one -> p (mc one)", p=P))
            mrow_s = cst.tile([LM, 1], I32, tag="mrow")
            nc.sync.dma_start(mrow_s[:], mrow[:])
            tgt_s = cst.tile([LM, 1], I32, tag="tgt")
            nc.sync.dma_start(tgt_s[:], tgtr[:])

            # dynamic offset of the peer half in cc_out
            pid = nc.sync.partition_id()
            off = (1 - pid % 2) * R

            # ---- persistent state ----
            xsaf = cst.tile([P, NT, E], F32, tag="xsaf")
            xsa16 = cst.tile([P, MC, E], BF, tag="xsa16")
            xsaT = cst.tile([P, EC, L], BF, tag="xsaT")
            zT = cst.tile([P, EC, R], BF, tag="zT")

            def norm_from(pin, fout, tt_bf=None):
                """fout = pin / (1 + std(pin, ddof=1)); optionally bf16 copy."""
                st6 = scp.tile([P, 6], F32, tag="st6")
                nc.vector.bn_stats(st6[:], pin)
                mv = scp.tile([P, 2], F32, tag="mv")
                nc.vector.bn_aggr(mv[:], st6[:])
                sd = scp.tile([P, 1], F32, tag="sd")
                nc.scalar.activation(sd[:], mv[:, 1:2], Sqrt, scale=float(E) / (E - 1))
                d1 = scp.tile([P, 1], F32, tag="d1")
                nc.vector.tensor_scalar_add(d1[:], sd[:], 1.0)
                rv = scp.tile([P, 1], F32, tag="rv")
                nc.vector.reciprocal(rv[:], d1[:])
                nc.scalar.activation(fout, pin, Ident, scale=rv[:, 0:1])
                if tt_bf is not None:
                    nc.vector.tensor_copy(tt_bf, fout)

            def dbg_out(ap):
                fo = scp.tile([1, 1], F32, tag="fout")
                nc.scalar.activation(fo[:], ap, Copy)
                nc.sync.dma_start(outv[:], fo[:])

            # ---- init: gather embeddings, norm, transpose ----
            for mc in range(MC):
                gz = scp.tile([P, E], F32, tag="gz")
                nc.gpsimd.indirect_dma_start(
                    out=gz[:], out_offset=None, in_=emb[:],
                    in_offset=bass.IndirectOffsetOnAxis(ap=zidx_s[:, mc:mc + 1], axis=0))
                if mc < NT:
                    norm_from(gz[:], xsaf[:, mc, :], xsa16[:, mc, :])
                else:
                    zf = scp.tile([P, E], F32, tag="zf")
                    norm_from(gz[:], zf[:], xsa16[:, mc, :])
            for ec in range(EC):
                for mc in range(MC):
                    tp = pst.tile([P, P], BF, tag="t")
                    nc.tensor.transpose(
                        tp[:], xsa16[:, mc, ec * P:(ec + 1) * P], ident_bf[:])
                    nc.vector.tensor_copy(xsaT[:, ec, mc * P:(mc + 1) * P], tp[:])
                nc.vector.tensor_copy(zT[:, ec, :], xsaT[:, ec, 0:R])

            if STAGE <= 0:
                dbg_out(xsaT[0:1, 0, 0:1])
                return nc
            # ================= layers =================
            for d in range(N_LAYERS):
                # --- load weights for this layer ---
                wt_s = wsm.tile([P, EC, E], BF, tag="wt")
                nc.sync.dma_start(wt_s[:], wts[d])
                wtt_s = wsm.tile([P, EC, E], BF, tag="wtt")
                nc.sync.dma_start(wtt_s[:], wtts[d])
                wtc_s = wsm.tile([P, EC, E], BF, tag="wtc")
                nc.sync.dma_start(wtc_s[:], wtcs[d])
                wtct_s = wsm.tile([P, EC, E], BF, tag="wtct")
                nc.sync.dma_start(wtct_s[:], wtcts[d])
                wut_s = wsm.tile([P, EC, E], BF, tag="wut")
                nc.sync.dma_start(wut_s[:], wuts[d])
                bt_s = wsm.tile([1, E], BF, tag="bt")
                nc.sync.dma_start(bt_s[:], bts[d])
                wq_s = wbig.tile([P, EC, KE], BF, tag="wq")
                nc.sync.dma_start(wq_s[:], wqt[d])
                wo_s = wbig.tile([P, KC, E], BF, tag="wo")
                nc.sync.dma_start(wo_s[:], wov[d])

                # --- transitions (transposed pre-relu) ---
                # a1T = (roll(xsa,+1)[own] @ Wt).T ; b1T = (roll(xsa,-1)[own] @ Wtc.T).T
                a1rT = actp.tile([P, EC, R], BF, tag="a1rT")
                b1rT = actp.tile([P, EC, R], BF, tag="b1rT")
                # roll(+1): local src cols {511, 0..254}; roll(-1): {1..255, 256}
                for dst, wmat, pieces in (
                    (a1rT, wt_s, (((511, 512), (0, 1)), ((0, 255), (1, 256)))),
                    (b1rT, wtct_s, (((1, 256), (0, 255)), ((256, 257), (255, 256)))),
                ):
                    for e2t in range(EC):
                        ps = pst.tile([P, R], F32, tag="t")
                        nmm = EC * len(pieces)
                        i = 0
                        for ec in range(EC):
                            for (s0, s1), (d0, d1) in pieces:
                                nc.tensor.matmul(
                                    ps[:, d0:d1],
                                    wmat[:, ec, e2t * P:(e2t + 1) * P],
                                    xsaT[:, ec, s0:s1],
                                    start=(i == 0), stop=(i == nmm - 1))
                                i += 1
                        nc.scalar.activation(dst[:, e2t, :], ps[:], Relu)

                # --- xsad = a1r@Wtc + b1r@Wt.T + z@Wu.T + bt ---
                xsad_s = actp.tile([P, NT, E], F32, tag="xsad")
                for tt in range(NT):
                    ps = psh.tile([P, E], F32, tag="h")
                    first = True
                    for e2c in range(EC):
                        nc.tensor.matmul(ps[:], a1rT[:, e2c, tt * P:(tt + 1) * P],
                                         wtc_s[:, e2c, :], start=first, stop=False)
                        first = False
                    for e2c in range(EC):
                        nc.tensor.matmul(ps[:], b1rT[:, e2c, tt * P:(tt + 1) * P],
                                         wtt_s[:, e2c, :], start=False, stop=False)
                    for ec in range(EC):
                        nc.tensor.matmul(ps[:], zT[:, ec, tt * P:(tt + 1) * P],
                                         wut_s[:, ec, :], start=False, stop=False)
                    nc.tensor.matmul(ps[:], ones_row_bf[0:1, :], bt_s[0:1, :],
                                     start=False, stop=True)
                    nc.vector.tensor_copy(xsad_s[:, tt, :], ps[:])

                # --- q (transposed): qT = Wq @ xsaT_own ---
                qT = actp.tile([P, KC, R], BF, tag="qT")
                for jc in range(KC):
                    ps = pst.tile([P, R], F32, tag="t")
                    for ec in range(EC):
                        nc.tensor.matmul(ps[:], wq_s[:, ec, jc * P:(jc + 1) * P],
                                         xsaT[:, ec, 0:R],
                                         start=(ec == 0), stop=(ec == EC - 1))
                    nc.vector.tensor_copy(qT[:, jc, :], ps[:])

                if STAGE <= 1:
                    dbg_out(qT[0:1, 0, 0:1])
                    return nc
                # --- attention scores + exp (softmax numerator) ---
                expT = actp.tile([P, MC, K, R], BF, tag="expT")
                for k in range(K):
                    for mc in range(MC):
                        ps = pst.tile([P, R], F32, tag="t")
                        for ec in range(EC):
                            nc.tensor.matmul(
                                ps[:], xsaT[:, ec, mc * P:(mc + 1) * P],
                                qT[:, 2 * k + ec, :],
                                start=(ec == 0), stop=(ec == EC - 1))
                        nc.scalar.activation(expT[:, mc, k, :], ps[:], Exp,
                                             scale=1.0 / np.sqrt(E))

                # --- per head-pair: sums, recip, values, scale ---
                yT = actp.tile([P, KC, R], BF, tag="yT")
                for pr in range(K // 2):
                    sps = psr.tile([1, 2 * R], F32, tag="r")
                    for mc in range(MC):
                        nc.tensor.matmul(
                            sps[:], ones_col_bf[:, 0:1],
                            expT[:, mc, 2 * pr:2 * pr + 2, :],
                            start=(mc == 0), stop=(mc == MC - 1))
                    rc = scp.tile([1, 2 * R], BF, tag="recip")
                    with nc.allow_low_precision("bf16 softmax recip broadcast"):
                        nc.vector.reciprocal(rc[:], sps[:])
                    rps = psr.tile([P, 2 * R], F32, tag="r")
                    nc.tensor.matmul(
                        rps[:], ones_row_bf[0:1, :], rc[0:1, :],
                        start=True, stop=True)
                    rsb = scp.tile([P, 2 * R], F32, tag="rsb")
                    nc.vector.tensor_copy(rsb[:], rps[:])
                    for ec in range(EC):
                        yps = psy.tile([P, 2 * R], F32, tag="y")
                        for mc in range(MC):
                            nc.tensor.matmul(
                                yps[:], xsa16[:, mc, ec * P:(ec + 1) * P],
                                expT[:, mc, 2 * pr:2 * pr + 2, :],
                                start=(mc == 0), stop=(mc == MC - 1))
                        for i in range(2):
                            nc.vector.tensor_tensor(
                                yT[:, (2 * pr + i) * 2 + ec, :],
                                yps[:, i * R:(i + 1) * R],
                                rsb[:, i * R:(i + 1) * R], MULT)

                if STAGE <= 2:
                    dbg_out(yT[0:1, 0, 0:1])
                    return nc
                # --- xid1T = relu(y @ Wd.T).T  [KE, t] ---
                xid1T = actp.tile([P, KC, R], BF, tag="xid1T")
                for ng in range(4):
                    wd_s = wdp.tile([P, KC, KE // 4], BF, tag="wd")
                    nc.sync.dma_start(wd_s[:], wdt[d, :, :, ng * (KE // 4):(ng + 1) * (KE // 4)])
                    for nt in range(4):
                        ps = pst.tile([P, R], F32, tag="t")
                        for kc in range(KC):
                            nc.tensor.matmul(
                                ps[:], wd_s[:, kc, nt * P:(nt + 1) * P],
                                yT[:, kc, :], start=(kc == 0), stop=(kc == KC - 1))
                        nc.scalar.activation(xid1T[:, ng * 4 + nt, :], ps[:], Relu)

                # --- xid = xid1 @ Wo, then norms ---
                d_cc_in, d_cc_out = cc_in[d % 2], cc_out[d % 2]
                xsaf_new = stp.tile([P, NT, E], F32, tag="xsafn")
                xsa16_new = stp.tile([P, MC, E], BF, tag="xsa16n")
                for tt in range(NT):
                    ps = psh.tile([P, E], F32, tag="h")
                    for kc in range(KC):
                        nc.tensor.matmul(ps[:], xid1T[:, kc, tt * P:(tt + 1) * P],
                                         wo_s[:, kc, :], start=(kc == 0),
                                         stop=(kc == KC - 1))
                    # v = xsad + xid
                    v_s = scp.tile([P, E], F32, tag="v")
                    nc.vector.tensor_tensor(v_s[:], ps[:], xsad_s[:, tt, :], ADD)
                    # norm(v), folded: w = xsa + 0.05 * v/(1+std(v))
                    st6 = scp.tile([P, 6], F32, tag="st6")
                    nc.vector.bn_stats(st6[:], v_s[:])
                    mv = scp.tile([P, 2], F32, tag="mv")
                    nc.vector.bn_aggr(mv[:], st6[:])
                    sd = scp.tile([P, 1], F32, tag="sd")
                    nc.scalar.activation(sd[:], mv[:, 1:2], Sqrt,
                                         scale=float(E) / (E - 1))
                    d20 = scp.tile([P, 1], F32, tag="d1")
                    nc.vector.tensor_scalar(d20[:], sd[:], 1.0 / STEP, 1.0 / STEP,
                                            MULT, ADD)
                    rv = scp.tile([P, 1], F32, tag="rv")
                    nc.vector.reciprocal(rv[:], d20[:])
                    w_s = scp.tile([P, E], F32, tag="w")
                    nc.vector.scalar_tensor_tensor(
                        w_s[:], v_s[:], rv[:, 0:1], xsaf[:, tt, :], MULT, ADD)
                    # xsa_new = w / (1+std(w))
                    norm_from(w_s[:], xsaf_new[:, tt, :], xsa16_new[:, tt, :])
                    nc.sync.dma_start(d_cc_in[tt * P:(tt + 1) * P, :],
                                      xsa16_new[:, tt, :])

                if STAGE <= 3:
                    dbg_out(xsaf_new[0:1, 0, 0:1])
                    return nc
                # --- exchange halves ---
                nc.gpsimd.collective_compute(
                    "AllGather", mybir.AluOpType.bypass, replica_groups=groups,
                    ins=[d_cc_in[:]], outs=[d_cc_out[:]])
                for j in range(NT):
                    nc.sync.dma_start(xsa16_new[:, NT + j, :],
                                      d_cc_out[bass.ds(off + j * P, P), :])

                # --- transpose to xsaT ---
                xsaT_new = stp.tile([P, EC, L], BF, tag="xsaTn")
                for ec in range(EC):
                    for mc in range(MC):
                        tp = pst.tile([P, P], BF, tag="t")
                        nc.tensor.transpose(
                            tp[:], xsa16_new[:, mc, ec * P:(ec + 1) * P], ident_bf[:])
                        nc.vector.tensor_copy(
                            xsaT_new[:, ec, mc * P:(mc + 1) * P], tp[:])
                xsaf, xsa16, xsaT = xsaf_new, xsa16_new, xsaT_new

            if STAGE <= 4:
                dbg_out(xsaT[0:1, 0, 0:1])
                return nc
            # ================= head =================
            wkc_s = wbig.tile([P, EC, KN * E], BF, tag="wq")
            nc.sync.dma_start(wkc_s[:], wkct[:])
            bkc_s = wsm.tile([1, KN * E], BF, tag="bkc")
            nc.sync.dma_start(bkc_s[:], bkcr[:])
            wem_s = wsm.tile([P, EC, E], BF, tag="wem")
            nc.sync.dma_start(wem_s[:], wem[:])

            d_cc_out = cc_out[(N_LAYERS - 1) % 2]
            # lptok gather (bf16 rows from the last AllGather), transposed
            gl = scp.tile([LM, E], BF, tag="gl")
            nc.gpsimd.indirect_dma_start(
                out=gl[:], out_offset=None, in_=d_cc_out[:],
                in_offset=bass.IndirectOffsetOnAxis(ap=mrow_s[:, 0:1], axis=0))
            lptokT = scp.tile([P, EC, LM], BF, tag="lptokT")
            for ec in range(EC):
                tp = pst.tile([P, P], BF, tag="t")
                nc.tensor.transpose(tp[:, 0:LM], gl[:, ec * P:(ec + 1) * P],
                                    ident_bf[0:LM, 0:LM])
                nc.vector.tensor_copy(lptokT[:, ec, :], tp[:, 0:LM])

            # xxT[e', n] with n = k*64 + lm (k-major)
            xxT = scp.tile([P, EC, KN, LM], BF, tag="xxT")
            for kk in range(KN):
                for ept in range(EC):
                    ps = pst.tile([P, LM], F32, tag="t")
                    for ec in range(EC):
                        nc.tensor.matmul(
                            ps[:], wkc_s[:, ec, kk * E + ept * P:kk * E + (ept + 1) * P],
                            lptokT[:, ec, :], start=(ec == 0), stop=False)
                    nc.tensor.matmul(
                        ps[:], bkc_s[0:1, kk * E + ept * P:kk * E + (ept + 1) * P],
                        ones_row_bf[0:1, 0:LM], start=False, stop=True)
                    nc.vector.tensor_copy(xxT[:, ept, kk, :], ps[:])

            NH = KN * LM  # 256 head rows
            # t1T[l, n] = xsa[l,:] . xx[n,:]
            t1T = scp.tile([P, MC, NH], BF, tag="t1T")
            for lc in range(MC):
                ps = pst.tile([P, NH], F32, tag="t")
                for ept in range(EC):
                    nc.tensor.matmul(ps[:], xsaT[:, ept, lc * P:(lc + 1) * P],
                                     xxT[:, ept, :, :], start=(ept == 0),
                                     stop=(ept == EC - 1))
                nc.vector.tensor_copy(t1T[:, lc, :], ps[:])
            # t2T[e, n] = sum_l xsa[l, e] t1[n, l]
            t2T = scp.tile([P, EC, NH], BF, tag="t2T")
            for ec in range(EC):
                ps = pst.tile([P, NH], F32, tag="t")
                for lc in range(MC):
                    nc.tensor.matmul(ps[:], xsa16[:, lc, ec * P:(ec + 1) * P],
                                     t1T[:, lc, :], start=(lc == 0),
                                     stop=(lc == MC - 1))
                nc.vector.tensor_copy(t2T[:, ec, :], ps[:])
            # sT[e2, n] = Wem.T @ t2  (s = t2 @ Wem)
            sT = scp.tile([P, EC, NH], BF, tag="sT")
            for e2t in range(EC):
                ps = pst.tile([P, NH], F32, tag="t")
                for ec in range(EC):
                    nc.tensor.matmul(ps[:], wem_s[:, ec, e2t * P:(e2t + 1) * P],
                                     t2T[:, ec, :], start=(ec == 0),
                                     stop=(ec == EC - 1))
                nc.vector.tensor_copy(sT[:, e2t, :], ps[:])

            if STAGE <= 5:
                dbg_out(sT[0:1, 0, 0:1])
                return nc
            # logits over the vocab half: exp-sum accumulation
            esums = cst.tile([P, NH // P, GC], F32, tag="esums")
            for gc in range(GC):
                et = wdp.tile([P, EC, GCW], BF, tag="wd")
                nc.sync.dma_start(et[:], embT[:, :, gc * GCW:(gc + 1) * GCW])
                for ntl in range(NH // P):
                    ps = pst.tile([P, GCW], F32, tag="t")
                    for e2c in range(EC):
                        nc.tensor.matmul(
                            ps[:], sT[:, e2c, ntl * P:(ntl + 1) * P],
                            et[:, e2c, :], start=(e2c == 0), stop=(e2c == EC - 1))
                    junk = scp.tile([P, GCW], BF, tag="junk")
                    nc.scalar.activation(junk[:], ps[:], Exp,
                                         accum_out=esums[:, ntl, gc:gc + 1])
            Sh = scp.tile([P, NH // P, 1], F32, tag="Sh")
            for ntl in range(NH // P):
                nc.vector.reduce_sum(Sh[:, ntl, :], esums[:, ntl, :],
                                     axis=mybir.AxisListType.X)
                nc.sync.dma_start(st_in[ntl * P:(ntl + 1) * P, :], Sh[:, ntl, :])
            nc.gpsimd.collective_compute(
                "AllGather", mybir.AluOpType.bypass, replica_groups=groups,
                ins=[st_in[:]], outs=[st_out[:]])
            st2 = scp.tile([P, NH // P, 2], F32, tag="st2")
            for rr in range(2):
                nc.sync.dma_start(
                    st2[:, :, rr],
                    st_out[rr * R:(rr + 1) * R, :].rearrange(
                        "(nt p) one -> p (nt one)", p=P))
            stot = scp.tile([P, NH // P], F32, tag="stot")
            nc.vector.tensor_tensor(stot[:], st2[:, :, 0], st2[:, :, 1], ADD)
            lse = scp.tile([P, NH // P], F32, tag="lse")
            nc.scalar.activation(lse[:], stot[:], Ln)

            if STAGE <= 6:
                dbg_out(lse[0:1, 0:1])
                return nc
            # target logits: gather embed rows of targets, transpose
            gt = scp.tile([LM, E], F32, tag="gt")
            nc.gpsimd.indirect_dma_start(
                out=gt[:], out_offset=None, in_=emb[:],
                in_offset=bass.IndirectOffsetOnAxis(ap=tgt_s[:, 0:1], axis=0))
            gt16 = scp.tile([LM, E], BF, tag="gt16")
            nc.vector.tensor_copy(gt16[:], gt[:])
            ett = scp.tile([P, EC, LM], BF, tag="ett")
            for e2c in range(EC):
                tp = pst.tile([P, P], BF, tag="t")
                nc.tensor.transpose(tp[:, 0:LM], gt16[:, e2c * P:(e2c + 1) * P],
                                    ident_bf[0:LM, 0:LM])
                nc.vector.tensor_copy(ett[:, e2c, :], tp[:, 0:LM])
            if STAGE <= 61:
                dbg_out(ett[0:1, 0, 0:1])
                return nc
            tlog = scp.tile([P, NH // P], F32, tag="tlog")
            for ntl in range(NH // P):
                ps = pst.tile([P, LM], F32, tag="t")
                for e2c in range(EC):
                    nc.tensor.matmul(ps[:], sT[:, e2c, ntl * P:(ntl + 1) * P],
                                     ett[:, e2c, :], start=(e2c == 0),
                                     stop=(e2c == EC - 1))
                junk2 = scp.tile([P, LM], F32, tag="junk2")
                nc.vector.tensor_tensor(junk2[:], ps[:], imask[:], MULT)
                nc.vector.reduce_sum(tlog[:, ntl:ntl + 1], junk2[:],
                                     axis=mybir.AxisListType.X)

            if STAGE <= 62:
                dbg_out(tlog[0:1, 0:1])
                return nc
            # cent: logsumexp over k (4 rows per lm spread across partitions)
            xs_ = scp.tile([P, NH // P], F32, tag="xs_")
            nc.vector.tensor_tensor(xs_[:], tlog[:], lse[:],
                                    mybir.AluOpType.subtract)
            ex_ = scp.tile([P, NH // P], F32, tag="ex_")
            nc.scalar.activation(ex_[:], xs_[:], Exp)
            if STAGE <= 63:
                dbg_out(ex_[0:1, 0:1])
                return nc
            kps = psh.tile([LM, NH // P], F32, tag="h")
            nc.tensor.matmul(kps[:], imask[:], ex_[:], start=True, stop=True)
            ksum = scp.tile([LM, 1], F32, tag="ksum")
            nc.vector.reduce_sum(ksum[:], kps[:, 0:2], axis=mybir.AxisListType.X)
            cent = scp.tile([LM, 1], F32, tag="cent")
            nc.scalar.activation(cent[:], ksum[:], Ln, scale=1.0 / KN)
            if STAGE <= 64:
                dbg_out(cent[0:1, 0:1])
                return nc
            fps = psh.tile([1, 1], F32, tag="h")
            nc.tensor.matmul(fps[:], ones_col_f[0:LM, 0:1], cent[:, 0:1],
                             start=True, stop=True)
            fout = scp.tile([1, 1], F32, tag="fout")
            nc.scalar.activation(fout[:], fps[:], Copy, scale=-1.0 / LM)
            nc.sync.dma_start(outv[:], fout[:])

    nc.compile()
    nc._kernel_compiled = True
    return nc


def _build_wrapper():
    nc = _build()
    if not getattr(nc, "_kernel_compiled", False):
        nc.compile()
    return nc


_CACHE = {}


def _get_nc():
    if "nc" not in _CACHE:
        _CACHE["nc"] = _build_wrapper()
    return _CACHE["nc"]


def _chunk_pe(w):
    """[rows, cols] -> [128, rows//128, cols] (partition-chunked)."""
    r, c = w.shape
    return np.ascontiguousarray(w.reshape(r // P, P, c).swapaxes(0, 1))


def kernel(**inputs):
    nc = _get_nc()
    masked = np.asarray(inputs["masked"]).astype(np.int64)
    unmasked = np.asarray(inputs["unmasked"]).astype(np.int64)
    mask = np.asarray(inputs["mask"]).astype(np.int64)
    embed = np.asarray(inputs["embed"], dtype=np.float32)
    Wt, bt, Wtc = (np.asarray(inputs[k], dtype=np.float32) for k in ("Wt", "bt", "Wtc"))
    Wq, Wd, Wo, Wu = (np.asarray(inputs[k], dtype=np.float32) for k in ("Wq", "Wd", "Wo", "Wu"))
    Wem, Wkc, bkc = (np.asarray(inputs[k], dtype=np.float32) for k in ("Wem", "Wkc", "bkc"))

    embT = embed.T  # [E, G]
    shared = {
        "emb": embed,
        "wdt": np.stack([_chunk_pe(Wd[d].T).astype(bf16) for d in range(D)]),
        "wqt": np.stack([_chunk_pe(Wq[d].T).astype(bf16) for d in range(D)]),
        "wov": np.stack([_chunk_pe(Wo[d]).astype(bf16) for d in range(D)]),
        "wts": np.stack([_chunk_pe(Wt[d]).astype(bf16) for d in range(D)]),
        "wtts": np.stack([_chunk_pe(Wt[d].T).astype(bf16) for d in range(D)]),
        "wtcs": np.stack([_chunk_pe(Wtc[d]).astype(bf16) for d in range(D)]),
        "wtcts": np.stack([_chunk_pe(Wtc[d].T).astype(bf16) for d in range(D)]),
        "wuts": np.stack([_chunk_pe(Wu[d].T).astype(bf16) for d in range(D)]),
        "bts": bt.astype(bf16).reshape(D, 1, E),
        "wkct": _chunk_pe(Wkc.T).astype(bf16),
        "bkcr": bkc.astype(bf16).reshape(1, KN * E),
        "wem": _chunk_pe(Wem).astype(bf16),
        "imaskd": np.tile(np.eye(LM, dtype=np.float32), (P // LM, 1)),
    }
    tgt = np.take_along_axis(unmasked, mask, axis=1)  # [B, LM]

    in_maps = []
    for c in range(8):
        b, h = c // 2, c % 2
        local = np.concatenate(
            [masked[b, h * R:(h + 1) * R], masked[b, (1 - h) * R:(2 - h) * R]])
        m = dict(shared)
        m["embT"] = _chunk_pe(embT[:, h * GH:(h + 1) * GH]).astype(bf16)
        m["zidx"] = local.astype(np.int32).reshape(L, 1)
        m["mrow"] = mask[b].astype(np.int32).reshape(LM, 1)
        m["tgtr"] = tgt[b].astype(np.int32).reshape(LM, 1)
        in_maps.append(m)

    _CACHE["in_maps"] = in_maps
    res = run_bass_kernel_spmd(nc, in_maps, list(range(8)))
    out = np.array([res.results[2 * b]["out"][0, 0] for b in range(B)],
                   dtype=np.float32)
    return out


if __name__ == "__main__":
    ins = dict(np.load("/tmp/inputs.npz"))
    out = kernel(**ins)
    print("kernel out:", out)


# revision 18
# speedup vs baseline: 3.0733x; 3.0733x over previous
"""Trainium2 Bass kernel for nn_AddModelWithAttentionStacked.

Sharding (8 cores): core c handles batch b=c//2 and token-half h=c%2
(tokens [h*256, h*256+256) of L=512). The 6-layer attention stack runs
with per-pair AllGather of the updated xsa half each layer. The vocab
head (G=32000 logsumexp) is split across the pair (16000 columns each),
combined with a tiny stats AllGather. Per-batch results are read from
core 2b.

Token order on each core is LOCAL: [own half | other half]. Attention
and the head contractions are permutation-invariant over tokens, and the
roll(+-1) edge columns land at uniform local positions (511 / 256), so
one SPMD program serves all cores; the only data-dependent part is a
dynamic DMA offset (derived from partition_id) selecting the peer half
from the AllGather output.
"""

import numpy as np
import ml_dtypes

import concourse.bass as bass
import concourse.mybir as mybir
import concourse.tile as tile
from concourse import bacc
from concourse.bass_utils import run_bass_kernel_spmd
from concourse.masks import make_identity

bf16 = ml_dtypes.bfloat16
F32 = mybir.dt.float32
F32R = mybir.dt.float32r
BF = mybir.dt.bfloat16
I32 = mybir.dt.int32

P = 128
B, L, E, K, D, G, LM, KN = 4, 512, 256, 8, 6, 32000, 64, 4
R = L // 2          # own rows per core
GH = G // 2         # vocab half per core
KE = K * E          # 2048
NT = R // P         # 2  own-token tiles
MC = L // P         # 4  full-token tiles (local order)
EC = E // P         # 2  feature chunks
KC = KE // P        # 16 ke chunks
GC = 32             # vocab chunks per core
GCW = GH // GC      # 500 columns per vocab chunk
STEP = 0.05
Exp = mybir.ActivationFunctionType.Exp
Ln = mybir.ActivationFunctionType.Ln
Sqrt = mybir.ActivationFunctionType.Sqrt
Ident = mybir.ActivationFunctionType.Identity
Relu = mybir.ActivationFunctionType.Relu
Copy = mybir.ActivationFunctionType.Copy
ADD = mybir.AluOpType.add
MULT = mybir.AluOpType.mult

import os
N_LAYERS = int(os.environ.get("KERNEL_LAYERS", D))  # dev knob
STAGE = int(os.environ.get("KERNEL_STAGE", 99))  # truncate build for HW bisect
NOCC = bool(int(os.environ.get("KERNEL_NOCC", "0")))  # replace collectives with local DMA (for TimelineSim)


def _build():
    nc = bacc.Bacc("TRN2", target_bir_lowering=False, debug=False,
                   enable_asserts=False, num_devices=8)

    # ---------------- inputs (per-core) ----------------
    emb = nc.dram_tensor("emb", [G, E], F32, kind="ExternalInput")
    embT = nc.dram_tensor("embT", [P, EC, GH], BF, kind="ExternalInput")
    wdt = nc.dram_tensor("wdt", [D, P, KC, KE], BF, kind="ExternalInput")
    wqt = nc.dram_tensor("wqt", [D, P, EC, KE], BF, kind="ExternalInput")
    wov = nc.dram_tensor("wov", [D, P, KC, E], BF, kind="ExternalInput")
    wts = nc.dram_tensor("wts", [D, P, EC, E], BF, kind="ExternalInput")
    wtts = nc.dram_tensor("wtts", [D, P, EC, E], BF, kind="ExternalInput")
    wtcs = nc.dram_tensor("wtcs", [D, P, EC, E], BF, kind="ExternalInput")
    wtcts = nc.dram_tensor("wtcts", [D, P, EC, E], BF, kind="ExternalInput")
    wuts = nc.dram_tensor("wuts", [D, P, EC, E], BF, kind="ExternalInput")
    bts = nc.dram_tensor("bts", [D, 1, E], BF, kind="ExternalInput")
    wkct = nc.dram_tensor("wkct", [P, EC, KN * E], BF, kind="ExternalInput")
    bkcr = nc.dram_tensor("bkcr", [1, KN * E], BF, kind="ExternalInput")
    wem = nc.dram_tensor("wem", [P, EC, E], BF, kind="ExternalInput")
    zidx = nc.dram_tensor("zidx", [L, 1], I32, kind="ExternalInput")
    mrow = nc.dram_tensor("mrow", [LM, 1], I32, kind="ExternalInput")
    tgtr = nc.dram_tensor("tgtr", [LM, 1], I32, kind="ExternalInput")
    imaskd = nc.dram_tensor("imaskd", [P, LM], F32, kind="ExternalInput")

    outv = nc.dram_tensor("out", [1, 1], F32, kind="ExternalOutput")

    # internal DRAM for collectives (2 alternating sets)
    cc_in = [nc.dram_tensor(f"cc_in{i}", [R, E], BF) for i in range(2)]
    cc_out = [nc.dram_tensor(f"cc_out{i}", [L, E], BF) for i in range(2)]
    st_in = nc.dram_tensor("st_in", [R, 1], F32)
    st_out = nc.dram_tensor("st_out", [L, 1], F32)
    groups = [[0, 1], [2, 3], [4, 5], [6, 7]]

    with tile.TileContext(nc) as tc:
        with (
            tc.tile_pool(name="cst", bufs=1) as cst,
            tc.tile_pool(name="wsm", bufs=2) as wsm,      # small weights
            tc.tile_pool(name="wbig", bufs=2) as wbig,    # wq/wo
            tc.tile_pool(name="wd", bufs=2) as wdp,       # wd chunks
            tc.tile_pool(name="state", bufs=2) as stp,    # xsa tiles
            tc.tile_pool(name="act", bufs=1) as actp,     # per-layer activations
            tc.tile_pool(name="sc", bufs=2) as scp,       # small scratch
            tc.tile_pool(name="pt", bufs=2, space="PSUM") as pst,   # [128,512] transient
            tc.tile_pool(name="ph", bufs=2, space="PSUM") as psh,   # [128,256] held
            tc.tile_pool(name="py", bufs=2, space="PSUM") as psy,   # [128,512] values
            tc.tile_pool(name="pr", bufs=2, space="PSUM") as psr,   # R / S
        ):
            # ---- constants ----
            ident_bf = cst.tile([P, P], BF, tag="ident")
            make_identity(nc, ident_bf[:])
            ones_col_bf = cst.tile([P, 1], BF, tag="onescb")
            nc.vector.memset(ones_col_bf[:], 1.0)
            ones_col_f = cst.tile([P, 1], F32, tag="onescf")
            nc.vector.memset(ones_col_f[:], 1.0)
            ones_row_f = cst.tile([1, P], F32, tag="onesrf")
            nc.vector.memset(ones_row_f[:], 1.0)
            ones_row_bf = cst.tile([1, P], BF, tag="onesrb")
            nc.vector.memset(ones_row_bf[:], 1.0)
            imask = cst.tile([P, LM], F32, tag="imask")
            nc.sync.dma_start(imask[:], imaskd[:])

            # index tensors to SBUF
            zidx_s = cst.tile([P, MC], I32, tag="zidx")
            nc.sync.dma_start(
                zidx_s[:], zidx.rearrange("(mc p) one -> p (mc one)", p=P))
            mrow_s = cst.tile([LM, 1], I32, tag="mrow")
            nc.sync.dma_start(mrow_s[:], mrow[:])
            tgt_s = cst.tile([LM, 1], I32, tag="tgt")
            nc.sync.dma_start(tgt_s[:], tgtr[:])

            # dynamic offset of the peer half in cc_out
            pid = nc.sync.partition_id()
            off = (1 - pid % 2) * R

            # ---- persistent state ----
            xsaf = cst.tile([P, NT, E], F32, tag="xsaf")
            xsa16 = cst.tile([P, MC, E], BF, tag="xsa16")
            xsaT = cst.tile([P, EC, L], BF, tag="xsaT")
            zT = cst.tile([P, EC, R], BF, tag="zT")

            def norm_from(pin, fout, tt_bf=None):
                """fout = pin / (1 + std(pin, ddof=1)); optionally bf16 copy."""
                st6 = scp.tile([P, 6], F32, tag="st6")
                nc.vector.bn_stats(st6[:], pin)
                mv = scp.tile([P, 2], F32, tag="mv")
                nc.vector.bn_aggr(mv[:], st6[:])
                sd = scp.tile([P, 1], F32, tag="sd")
                nc.scalar.activation(sd[:], mv[:, 1:2], Sqrt, scale=float(E) / (E - 1))
                d1 = scp.tile([P, 1], F32, tag="d1")
                nc.vector.tensor_scalar_add(d1[:], sd[:], 1.0)
                rv = scp.tile([P, 1], F32, tag="rv")
                nc.vector.reciprocal(rv[:], d1[:])
                nc.scalar.activation(fout, pin, Ident, scale=rv[:, 0:1])
                if tt_bf is not None:
                    nc.vector.tensor_copy(tt_bf, fout)

            def dbg_out(ap):
                fo = scp.tile([1, 1], F32, tag="fout")
                nc.scalar.activation(fo[:], ap, Copy)
                nc.sync.dma_start(outv[:], fo[:])

            # ---- init: gather embeddings, norm, transpose ----
            for mc in range(MC):
                gz = scp.tile([P, E], F32, tag="gz")
                nc.gpsimd.indirect_dma_start(
                    out=gz[:], out_offset=None, in_=emb[:],
                    in_offset=bass.IndirectOffsetOnAxis(ap=zidx_s[:, mc:mc + 1], axis=0))
                if mc < NT:
                    norm_from(gz[:], xsaf[:, mc, :], xsa16[:, mc, :])
                else:
                    zf = scp.tile([P, E], F32, tag="zf")
                    norm_from(gz[:], zf[:], xsa16[:, mc, :])
            for ec in range(EC):
                for mc in range(MC):
                    tp = pst.tile([P, P], BF, tag="t")
                    nc.tensor.transpose(
                        tp[:], xsa16[:, mc, ec * P:(ec + 1) * P], ident_bf[:])
                    nc.vector.tensor_copy(xsaT[:, ec, mc * P:(mc + 1) * P], tp[:])
                nc.vector.tensor_copy(zT[:, ec, :], xsaT[:, ec, 0:R])

            if STAGE <= 0:
                dbg_out(xsaT[0:1, 0, 0:1])
                return nc
            # ================= layers =================
            for d in range(N_LAYERS):
                # --- load weights for this layer ---
                wt_s = wsm.tile([P, EC, E], BF, tag="wt")
                nc.sync.dma_start(wt_s[:], wts[d])
                wtt_s = wsm.tile([P, EC, E], BF, tag="wtt")
                nc.sync.dma_start(wtt_s[:], wtts[d])
                wtc_s = wsm.tile([P, EC, E], BF, tag="wtc")
                nc.sync.dma_start(wtc_s[:], wtcs[d])
                wtct_s = wsm.tile([P, EC, E], BF, tag="wtct")
                nc.sync.dma_start(wtct_s[:], wtcts[d])
                wut_s = wsm.tile([P, EC, E], BF, tag="wut")
                nc.sync.dma_start(wut_s[:], wuts[d])
                bt_s = wsm.tile([1, E], BF, tag="bt")
                nc.sync.dma_start(bt_s[:], bts[d])
                wq_s = wbig.tile([P, EC, KE], BF, tag="wq")
                nc.sync.dma_start(wq_s[:], wqt[d])
                wo_s = wbig.tile([P, KC, E], BF, tag="wo")
                nc.sync.dma_start(wo_s[:], wov[d])

                # --- transitions (transposed pre-relu) ---
                # a1T = (roll(xsa,+1)[own] @ Wt).T ; b1T = (roll(xsa,-1)[own] @ Wtc.T).T
                a1rT = actp.tile([P, EC, R], BF, tag="a1rT")
                b1rT = actp.tile([P, EC, R], BF, tag="b1rT")
                # roll(+1): local src cols {511, 0..254}; roll(-1): {1..255, 256}
                for dst, wmat, pieces in (
                    (a1rT, wt_s, (((511, 512), (0, 1)), ((0, 255), (1, 256)))),
                    (b1rT, wtct_s, (((1, 256), (0, 255)), ((256, 257), (255, 256)))),
                ):
                    for e2t in range(EC):
                        ps = pst.tile([P, R], F32, tag="t")
                        nmm = EC * len(pieces)
                        i = 0
                        for ec in range(EC):
                            for (s0, s1), (d0, d1) in pieces:
                                nc.tensor.matmul(
                                    ps[:, d0:d1],
                                    wmat[:, ec, e2t * P:(e2t + 1) * P],
                                    xsaT[:, ec, s0:s1],
                                    start=(i == 0), stop=(i == nmm - 1))
                                i += 1
                        nc.scalar.activation(dst[:, e2t, :], ps[:], Relu)

                # --- xsad = a1r@Wtc + b1r@Wt.T + z@Wu.T + bt ---
                xsad_s = actp.tile([P, NT, E], F32, tag="xsad")
                for tt in range(NT):
                    ps = psh.tile([P, E], F32, tag="h")
                    first = True
                    for e2c in range(EC):
                        nc.tensor.matmul(ps[:], a1rT[:, e2c, tt * P:(tt + 1) * P],
                                         wtc_s[:, e2c, :], start=first, stop=False)
                        first = False
                    for e2c in range(EC):
                        nc.tensor.matmul(ps[:], b1rT[:, e2c, tt * P:(tt + 1) * P],
                                         wtt_s[:, e2c, :], start=False, stop=False)
                    for ec in range(EC):
                        nc.tensor.matmul(ps[:], zT[:, ec, tt * P:(tt + 1) * P],
                                         wut_s[:, ec, :], start=False, stop=False)
                    nc.tensor.matmul(ps[:], ones_row_bf[0:1, :], bt_s[0:1, :],
                                     start=False, stop=True)
                    nc.vector.tensor_copy(xsad_s[:, tt, :], ps[:])

                # --- q (transposed): qT = Wq @ xsaT_own ---
                qT = actp.tile([P, KC, R], BF, tag="qT")
                for jc in range(KC):
                    ps = pst.tile([P, R], F32, tag="t")
                    for ec in range(EC):
                        nc.tensor.matmul(ps[:], wq_s[:, ec, jc * P:(jc + 1) * P],
                                         xsaT[:, ec, 0:R],
                                         start=(ec == 0), stop=(ec == EC - 1))
                    nc.vector.tensor_copy(qT[:, jc, :], ps[:])

                if STAGE <= 1:
                    dbg_out(qT[0:1, 0, 0:1])
                    return nc
                # --- attention scores + exp (softmax numerator) ---
                expT = actp.tile([P, MC, K, R], BF, tag="expT")
                for k in range(K):
                    for mc in range(MC):
                        ps = pst.tile([P, R], F32, tag="t")
                        for ec in range(EC):
                            nc.tensor.matmul(
                                ps[:], xsaT[:, ec, mc * P:(mc + 1) * P],
                                qT[:, 2 * k + ec, :],
                                start=(ec == 0), stop=(ec == EC - 1))
                        nc.scalar.activation(expT[:, mc, k, :], ps[:], Exp,
                                             scale=1.0 / np.sqrt(E))

                # --- per head-pair: sums, recip, values, scale ---
                yT = actp.tile([P, KC, R], BF, tag="yT")
                for pr in range(K // 2):
                    sps = psr.tile([1, 2 * R], F32, tag="r")
                    for mc in range(MC):
                        nc.tensor.matmul(
                            sps[:], ones_col_bf[:, 0:1],
                            expT[:, mc, 2 * pr:2 * pr + 2, :],
                            start=(mc == 0), stop=(mc == MC - 1))
                    rc = scp.tile([1, 2 * R], BF, tag="recip")
                    with nc.allow_low_precision("bf16 softmax recip broadcast"):
                        nc.vector.reciprocal(rc[:], sps[:])
                    rps = psr.tile([P, 2 * R], F32, tag="r")
                    nc.tensor.matmul(
                        rps[:], ones_row_bf[0:1, :], rc[0:1, :],
                        start=True, stop=True)
                    rsb = scp.tile([P, 2 * R], F32, tag="rsb")
                    nc.vector.tensor_copy(rsb[:], rps[:])
                    for ec in range(EC):
                        yps = psy.tile([P, 2 * R], F32, tag="y")
                        for mc in range(MC):
                            nc.tensor.matmul(
                                yps[:], xsa16[:, mc, ec * P:(ec + 1) * P],
                                expT[:, mc, 2 * pr:2 * pr + 2, :],
                                start=(mc == 0), stop=(mc == MC - 1))
                        for i in range(2):
                            nc.vector.tensor_tensor(
                                yT[:, (2 * pr + i) * 2 + ec, :],
                                yps[:, i * R:(i + 1) * R],
                                rsb[:, i * R:(i + 1) * R], MULT)

                if STAGE <= 2:
                    dbg_out(yT[0:1, 0, 0:1])
                    return nc
                # --- xid1T = relu(y @ Wd.T).T  [KE, t] ---
                xid1T = actp.tile([P, KC, R], BF, tag="xid1T")
                for ng in range(4):
                    wd_s = wdp.tile([P, KC, KE // 4], BF, tag="wd")
                    nc.sync.dma_start(wd_s[:], wdt[d, :, :, ng * (KE // 4):(ng + 1) * (KE // 4)])
                    for nt in range(4):
                        ps = pst.tile([P, R], F32, tag="t")
                        for kc in range(KC):
                            nc.tensor.matmul(
                                ps[:], wd_s[:, kc, nt * P:(nt + 1) * P],
                                yT[:, kc, :], start=(kc == 0), stop=(kc == KC - 1))
                        nc.scalar.activation(xid1T[:, ng * 4 + nt, :], ps[:], Relu)

                # --- xid = xid1 @ Wo, then norms ---
                d_cc_in, d_cc_out = cc_in[d % 2], cc_out[d % 2]
                xsaf_new = stp.tile([P, NT, E], F32, tag="xsafn")
                xsa16_new = stp.tile([P, MC, E], BF, tag="xsa16n")
                for tt in range(NT):
                    ps = psh.tile([P, E], F32, tag="h")
                    for kc in range(KC):
                        nc.tensor.matmul(ps[:], xid1T[:, kc, tt * P:(tt + 1) * P],
                                         wo_s[:, kc, :], start=(kc == 0),
                                         stop=(kc == KC - 1))
                    # v = xsad + xid
                    v_s = scp.tile([P, E], F32, tag="v")
                    nc.vector.tensor_tensor(v_s[:], ps[:], xsad_s[:, tt, :], ADD)
                    # norm(v), folded: w = xsa + 0.05 * v/(1+std(v))
                    st6 = scp.tile([P, 6], F32, tag="st6")
                    nc.vector.bn_stats(st6[:], v_s[:])
                    mv = scp.tile([P, 2], F32, tag="mv")
                    nc.vector.bn_aggr(mv[:], st6[:])
                    sd = scp.tile([P, 1], F32, tag="sd")
                    nc.scalar.activation(sd[:], mv[:, 1:2], Sqrt,
                                         scale=float(E) / (E - 1))
                    d20 = scp.tile([P, 1], F32, tag="d1")
                    nc.vector.tensor_scalar(d20[:], sd[:], 1.0 / STEP, 1.0 / STEP,
                                            MULT, ADD)
                    rv = scp.tile([P, 1], F32, tag="rv")
                    nc.vector.reciprocal(rv[:], d20[:])
                    w_s = scp.tile([P, E], F32, tag="w")
                    nc.vector.scalar_tensor_tensor(
                        w_s[:], v_s[:], rv[:, 0:1], xsaf[:, tt, :], MULT, ADD)
                    # xsa_new = w / (1+std(w))
                    norm_from(w_s[:], xsaf_new[:, tt, :], xsa16_new[:, tt, :])
                    nc.sync.dma_start(d_cc_in[tt * P:(tt + 1) * P, :],
                                      xsa16_new[:, tt, :])

                if STAGE <= 3:
                    dbg_out(xsaf_new[0:1, 0, 0:1])
                    return nc
                # --- exchange halves ---
                if NOCC:
                    nc.sync.dma_start(d_cc_out[0:R, :], d_cc_in[:])
                    nc.sync.dma_start(d_cc_out[R:2 * R, :], d_cc_in[:])
                else:
                    nc.gpsimd.collective_compute(
                        "AllGather", mybir.AluOpType.bypass, replica_groups=groups,
                        ins=[d_cc_in[:]], outs=[d_cc_out[:]])
                for j in range(NT):
                    nc.sync.dma_start(xsa16_new[:, NT + j, :],
                                      d_cc_out[bass.ds(off + j * P, P), :])

                # --- transpose to xsaT ---
                xsaT_new = stp.tile([P, EC, L], BF, tag="xsaTn")
                for ec in range(EC):
                    for mc in range(MC):
                        tp = pst.tile([P, P], BF, tag="t")
                        nc.tensor.transpose(
                            tp[:], xsa16_new[:, mc, ec * P:(ec + 1) * P], ident_bf[:])
                        nc.vector.tensor_copy(
                            xsaT_new[:, ec, mc * P:(mc + 1) * P], tp[:])
                xsaf, xsa16, xsaT = xsaf_new, xsa16_new, xsaT_new

            if STAGE <= 4:
                dbg_out(xsaT[0:1, 0, 0:1])
                return nc
            # ================= head =================
            wkc_s = wbig.tile([P, EC, KN * E], BF, tag="wq")
            nc.sync.dma_start(wkc_s[:], wkct[:])
            bkc_s = wsm.tile([1, KN * E], BF, tag="bkc")
            nc.sync.dma_start(bkc_s[:], bkcr[:])
            wem_s = wsm.tile([P, EC, E], BF, tag="wem")
            nc.sync.dma_start(wem_s[:], wem[:])

            d_cc_out = cc_out[(N_LAYERS - 1) % 2]
            # lptok gather (bf16 rows from the last AllGather), transposed
            gl = scp.tile([LM, E], BF, tag="gl")
            nc.gpsimd.indirect_dma_start(
                out=gl[:], out_offset=None, in_=d_cc_out[:],
                in_offset=bass.IndirectOffsetOnAxis(ap=mrow_s[:, 0:1], axis=0))
            lptokT = scp.tile([P, EC, LM], BF, tag="lptokT")
            for ec in range(EC):
                tp = pst.tile([P, P], BF, tag="t")
                nc.tensor.transpose(tp[:, 0:LM], gl[:, ec * P:(ec + 1) * P],
                                    ident_bf[0:LM, 0:LM])
                nc.vector.tensor_copy(lptokT[:, ec, :], tp[:, 0:LM])

            # xxT[e', n] with n = k*64 + lm (k-major)
            xxT = scp.tile([P, EC, KN, LM], BF, tag="xxT")
            for kk in range(KN):
                for ept in range(EC):
                    ps = pst.tile([P, LM], F32, tag="t")
                    for ec in range(EC):
                        nc.tensor.matmul(
                            ps[:], wkc_s[:, ec, kk * E + ept * P:kk * E + (ept + 1) * P],
                            lptokT[:, ec, :], start=(ec == 0), stop=False)
                    nc.tensor.matmul(
                        ps[:], bkc_s[0:1, kk * E + ept * P:kk * E + (ept + 1) * P],
                        ones_row_bf[0:1, 0:LM], start=False, stop=True)
                    nc.vector.tensor_copy(xxT[:, ept, kk, :], ps[:])

            NH = KN * LM  # 256 head rows
            # t1T[l, n] = xsa[l,:] . xx[n,:]
            t1T = scp.tile([P, MC, NH], BF, tag="t1T")
            for lc in range(MC):
                ps = pst.tile([P, NH], F32, tag="t")
                for ept in range(EC):
                    nc.tensor.matmul(ps[:], xsaT[:, ept, lc * P:(lc + 1) * P],
                                     xxT[:, ept, :, :], start=(ept == 0),
                                     stop=(ept == EC - 1))
                nc.vector.tensor_copy(t1T[:, lc, :], ps[:])
            # t2T[e, n] = sum_l xsa[l, e] t1[n, l]
            t2T = scp.tile([P, EC, NH], BF, tag="t2T")
            for ec in range(EC):
                ps = pst.tile([P, NH], F32, tag="t")
                for lc in range(MC):
                    nc.tensor.matmul(ps[:], xsa16[:, lc, ec * P:(ec + 1) * P],
                                     t1T[:, lc, :], start=(lc == 0),
                                     stop=(lc == MC - 1))
                nc.vector.tensor_copy(t2T[:, ec, :], ps[:])
            # sT[e2, n] = Wem.T @ t2  (s = t2 @ Wem)
            sT = scp.tile([P, EC, NH], BF, tag="sT")
            for e2t in range(EC):
                ps = pst.tile([P, NH], F32, tag="t")
                for ec in range(EC):
                    nc.tensor.matmul(ps[:], wem_s[:, ec, e2t * P:(e2t + 1) * P],
                                     t2T[:, ec, :], start=(ec == 0),
                                     stop=(ec == EC - 1))
                nc.vector.tensor_copy(sT[:, e2t, :], ps[:])

            if STAGE <= 5:
                dbg_out(sT[0:1, 0, 0:1])
                return nc
            # logits over the vocab half: exp-sum accumulation
            esums = cst.tile([P, NH // P, GC], F32, tag="esums")
            for gc in range(GC):
                et = wdp.tile([P, EC, GCW], BF, tag="wd")
                nc.sync.dma_start(et[:], embT[:, :, gc * GCW:(gc + 1) * GCW])
                for ntl in range(NH // P):
                    ps = pst.tile([P, GCW], F32, tag="t")
                    for e2c in range(EC):
                        nc.tensor.matmul(
                            ps[:], sT[:, e2c, ntl * P:(ntl + 1) * P],
                            et[:, e2c, :], start=(e2c == 0), stop=(e2c == EC - 1))
                    junk = scp.tile([P, GCW], BF, tag="junk")
                    nc.scalar.activation(junk[:], ps[:], Exp,
                                         accum_out=esums[:, ntl, gc:gc + 1])
            Sh = scp.tile([P, NH // P, 1], F32, tag="Sh")
            for ntl in range(NH // P):
                nc.vector.reduce_sum(Sh[:, ntl, :], esums[:, ntl, :],
                                     axis=mybir.AxisListType.X)
                nc.sync.dma_start(st_in[ntl * P:(ntl + 1) * P, :], Sh[:, ntl, :])
            if NOCC:
                nc.sync.dma_start(st_out[0:R, :], st_in[:])
                nc.sync.dma_start(st_out[R:2 * R, :], st_in[:])
            else:
                nc.gpsimd.collective_compute(
                    "AllGather", mybir.AluOpType.bypass, replica_groups=groups,
                    ins=[st_in[:]], outs=[st_out[:]])
            st2 = scp.tile([P, NH // P, 2], F32, tag="st2")
            for rr in range(2):
                nc.sync.dma_start(
                    st2[:, :, rr],
                    st_out[rr * R:(rr + 1) * R, :].rearrange(
                        "(nt p) one -> p (nt one)", p=P))
            stot = scp.tile([P, NH // P], F32, tag="stot")
            nc.vector.tensor_tensor(stot[:], st2[:, :, 0], st2[:, :, 1], ADD)
            lse = scp.tile([P, NH // P], F32, tag="lse")
            nc.scalar.activation(lse[:], stot[:], Ln)

            if STAGE <= 6:
                dbg_out(lse[0:1, 0:1])
                return nc
            # target logits: gather embed rows of targets, transpose
            gt = scp.tile([LM, E], F32, tag="gt")
            nc.gpsimd.indirect_dma_start(
                out=gt[:], out_offset=None, in_=emb[:],
                in_offset=bass.IndirectOffsetOnAxis(ap=tgt_s[:, 0:1], axis=0))
            gt16 = scp.tile([LM, E], BF, tag="gt16")
            nc.vector.tensor_copy(gt16[:], gt[:])
            ett = scp.tile([P, EC, LM], BF, tag="ett")
            for e2c in range(EC):
                tp = pst.tile([P, P], BF, tag="t")
                nc.tensor.transpose(tp[:, 0:LM], gt16[:, e2c * P:(e2c + 1) * P],
                                    ident_bf[0:LM, 0:LM])
                nc.vector.tensor_copy(ett[:, e2c, :], tp[:, 0:LM])
            if STAGE <= 61:
                dbg_out(ett[0:1, 0, 0:1])
                return nc
            tlog = scp.tile([P, NH // P], F32, tag="tlog")
            for ntl in range(NH // P):
                ps = pst.tile([P, LM], F32, tag="t")
                for e2c in range(EC):
                    nc.tensor.matmul(ps[:], sT[:, e2c, ntl * P:(ntl + 1) * P],
                                     ett[:, e2c, :], start=(e2c == 0),
                                     stop=(e2c == EC - 1))
                junk2 = scp.tile([P, LM], F32, tag="junk2")
                nc.vector.tensor_tensor(junk2[:], ps[:], imask[:], MULT)
                nc.vector.reduce_sum(tlog[:, ntl:ntl + 1], junk2[:],
                                     axis=mybir.AxisListType.X)

            if STAGE <= 62:
                dbg_out(tlog[0:1, 0:1])
                return nc
            # cent: logsumexp over k (4 rows per lm spread across partitions)
            xs_ = scp.tile([P, NH // P], F32, tag="xs_")
            nc.vector.tensor_tensor(xs_[:], tlog[:], lse[:],
                                    mybir.AluOpType.subtract)
            ex_ = scp.tile([P, NH // P], F32, tag="ex_")
            nc.scalar.activation(ex_[:], xs_[:], Exp)
            if STAGE <= 63:
                dbg_out(ex_[0:1, 0:1])
                return nc
            kps = psh.tile([LM, NH // P], F32, tag="h")
            nc.tensor.matmul(kps[:], imask[:], ex_[:], start=True, stop=True)
            ksum = scp.tile([LM, 1], F32, tag="ksum")
            nc.vector.reduce_sum(ksum[:], kps[:, 0:2], axis=mybir.AxisListType.X)
            cent = scp.tile([LM, 1], F32, tag="cent")
            nc.scalar.activation(cent[:], ksum[:], Ln, scale=1.0 / KN)
            if STAGE <= 64:
                dbg_out(cent[0:1, 0:1])
                return nc
            fps = psh.tile([1, 1], F32, tag="h")
            nc.tensor.matmul(fps[:], ones_col_f[0:LM, 0:1], cent[:, 0:1],
                             start=True, stop=True)
            fout = scp.tile([1, 1], F32, tag="fout")
            nc.scalar.activation(fout[:], fps[:], Copy, scale=-1.0 / LM)
            nc.sync.dma_start(outv[:], fout[:])

    nc.compile()
    nc._kernel_compiled = True
    return nc


def _build_wrapper():
    nc = _build()
    if not getattr(nc, "_kernel_compiled", False):
        nc.compile()
    return nc


_CACHE = {}


def _get_nc():
    if "nc" not in _CACHE:
        _CACHE["nc"] = _build_wrapper()
    return _CACHE["nc"]


def _chunk_pe(w):
    """[rows, cols] -> [128, rows//128, cols] (partition-chunked)."""
    r, c = w.shape
    return np.ascontiguousarray(w.reshape(r // P, P, c).swapaxes(0, 1))


def kernel(**inputs):
    nc = _get_nc()
    masked = np.asarray(inputs["masked"]).astype(np.int64)
    unmasked = np.asarray(inputs["unmasked"]).astype(np.int64)
    mask = np.asarray(inputs["mask"]).astype(np.int64)
    embed = np.asarray(inputs["embed"], dtype=np.float32)
    Wt, bt, Wtc = (np.asarray(inputs[k], dtype=np.float32) for k in ("Wt", "bt", "Wtc"))
    Wq, Wd, Wo, Wu = (np.asarray(inputs[k], dtype=np.float32) for k in ("Wq", "Wd", "Wo", "Wu"))
    Wem, Wkc, bkc = (np.asarray(inputs[k], dtype=np.float32) for k in ("Wem", "Wkc", "bkc"))

    embT = embed.T  # [E, G]
    shared = {
        "emb": embed,
        "wdt": np.stack([_chunk_pe(Wd[d].T).astype(bf16) for d in range(D)]),
        "wqt": np.stack([_chunk_pe(Wq[d].T).astype(bf16) for d in range(D)]),
        "wov": np.stack([_chunk_pe(Wo[d]).astype(bf16) for d in range(D)]),
        "wts": np.stack([_chunk_pe(Wt[d]).astype(bf16) for d in range(D)]),
        "wtts": np.stack([_chunk_pe(Wt[d].T).astype(bf16) for d in range(D)]),
        "wtcs": np.stack([_chunk_pe(Wtc[d]).astype(bf16) for d in range(D)]),
        "wtcts": np.stack([_chunk_pe(Wtc[d].T).astype(bf16) for d in range(D)]),
        "wuts": np.stack([_chunk_pe(Wu[d].T).astype(bf16) for d in range(D)]),
        "bts": bt.astype(bf16).reshape(D, 1, E),
        "wkct": _chunk_pe(Wkc.T).astype(bf16),
        "bkcr": bkc.astype(bf16).reshape(1, KN * E),
        "wem": _chunk_pe(Wem).astype(bf16),
        "imaskd": np.tile(np.eye(LM, dtype=np.float32), (P // LM, 1)),
    }
    tgt = np.take_along_axis(unmasked, mask, axis=1)  # [B, LM]

    in_maps = []
    for c in range(8):
        b, h = c // 2, c % 2
        local = np.concatenate(
            [masked[b, h * R:(h + 1) * R], masked[b, (1 - h) * R:(2 - h) * R]])
        m = dict(shared)
        m["embT"] = _chunk_pe(embT[:, h * GH:(h + 1) * GH]).astype(bf16)
        m["zidx"] = local.astype(np.int32).reshape(L, 1)
        m["mrow"] = mask[b].astype(np.int32).reshape(LM, 1)
        m["tgtr"] = tgt[b].astype(np.int32).reshape(LM, 1)
        in_maps.append(m)

    _CACHE["in_maps"] = in_maps
    res = run_bass_kernel_spmd(nc, in_maps, list(range(8)))
    out = np.array([res.results[2 * b]["out"][0, 0] for b in range(B)],
                   dtype=np.float32)
    return out


if __name__ == "__main__":
    ins = dict(np.load("/tmp/inputs.npz"))
    out = kernel(**ins)
    print("kernel out:", out)
